# revision 45
# baseline (speedup 1.0000x reference)
"""Attention-pooling kernel for TRN2 (8 NeuronCores, batch-parallel).

Computes, for x:[32,2048,1024], W:[1024,1024], b:[1024], ctx:[1024]:
    h = tanh(x @ W + b); scores = h . ctx
    weights = softmax(scores, axis=seq)
    out = sum_s weights[s] * x[s]          -> [32, 1024]

Sharding: data-parallel over batch, 4 batches per core.

x and W are cast to fp16 on the host (10-bit mantissa keeps the score
error at the f32r-baseline level) so the kernel can use the xbar DMA
transpose: each 512-row seq sub-tile of x is transposed HBM -> SBUF by
the DMA engines directly into a [128, 8, 512] tile with
xT[p, k, s] = x[s, 128k + p], i.e. each embed-block k is a contiguous
moving operand for the PE — no PE transposes and no PSUM-evacuation
copies for pass 1.

All heavy math stays on the PE: offloading the pooling to DVE was
tried and REGRESSED (~20% slower matmuls): big [128,512] DVE reads
contend with the PE's moving-operand SBUF fetches, so idle engines are
not free. Instead the softmax+pooling is ONLINE (flash-attention
style): each sub-tile's scores are transposed (4 tiny PE matmuls),
exponentiated against the sub-tile max (GPSIMD cross-partition max,
128-wide ACT exp), and pooled immediately (8 PE matmuls against a
straight fp16 copy of x), one sub-tile behind pass 1. A per-batch
scalar fixup r_t = exp(m_t - m)/Z rescales the four partial pools, so
nothing big serializes behind the last scores. Mid-stream PE
stationary/bank switches cost ~2x95ns on HW, so deferred work drains
as contiguous same-shape groups between pass-1 matmul groups.
"""

import numpy as np
from contextlib import ExitStack

import concourse.bacc as bacc
import concourse.mybir as mybir
import concourse.tile as tile
from concourse import masks
from concourse.bass_isa import ReduceOp
from concourse.bass_utils import run_bass_kernel_spmd

B, S, E, A = 32, 2048, 1024, 1024
NCORES = 8
BL = B // NCORES          # batches per core
ST = 512                  # seq sub-tile
NSUB = S // ST            # sub-tiles per batch
NCH = ST // 128           # 128-row s-chunks per sub-tile
KE = E // 128             # contraction chunks over embed dim
KA = A // 128             # chunks over attention dim

F32 = mybir.dt.float32
F16 = mybir.dt.float16
AX = mybir.AxisListType.X
AF = mybir.ActivationFunctionType
ALU = mybir.AluOpType


def _build(reps=1):
    nc = bacc.Bacc("TRN2", target_bir_lowering=False, debug=False,
                   num_devices=NCORES)
    x_d = nc.declare_dram_parameter("x", [BL * S, E], F16, isOutput=False)
    W_d = nc.declare_dram_parameter("W", [E, A], F16, isOutput=False)
    b_d = nc.declare_dram_parameter("b", [A], F32, isOutput=False)
    c_d = nc.declare_dram_parameter("ctx", [A], F16, isOutput=False)
    o_d = nc.declare_dram_parameter("out", [BL, E], F32, isOutput=True)

    with ExitStack() as ctx:
        tc = ctx.enter_context(tile.TileContext(nc))

        const_pool = ctx.enter_context(tc.tile_pool(name="const", bufs=1))
        xt_pool = ctx.enter_context(tc.tile_pool(name="xT", bufs=3))
        xb_pool = ctx.enter_context(tc.tile_pool(name="xb", bufs=5))
        h_pool = ctx.enter_context(tc.tile_pool(name="h", bufs=12))
        sc_pool = ctx.enter_context(tc.tile_pool(name="scores", bufs=2))
        sm_pool = ctx.enter_context(tc.tile_pool(name="softmax", bufs=2))
        pp_pool = ctx.enter_context(tc.tile_pool(name="parts", bufs=5))
        pt_pool = ctx.enter_context(tc.tile_pool(name="pT", bufs=3))
        out_pool = ctx.enter_context(tc.tile_pool(name="outs", bufs=2))

        ps_h = ctx.enter_context(tc.tile_pool(name="ps_h", bufs=2, space="PSUM"))
        ps_s = ctx.enter_context(tc.tile_pool(name="ps_s", bufs=2, space="PSUM"))
        ps_t = ctx.enter_context(tc.tile_pool(name="ps_t", bufs=1, space="PSUM"))
        ps_o = ctx.enter_context(tc.tile_pool(name="ps_o", bufs=1, space="PSUM"))

        # ---- constants ----
        ident = const_pool.tile([128, 128], F32)
        masks.make_identity(nc, ident[:])

        W_sb = const_pool.tile([128, KE * A], F16)
        b_sb = const_pool.tile([128, KA], F32)
        ctx_sb = const_pool.tile([128, KA], F16)

        tiles = [(rep, bi, t)
                 for rep in range(reps) for bi in range(BL) for t in range(NSUB)]

        def dma_xt(bi, t):
            # xbar DMA transpose: x rows [512, 1024] -> [128, 8, 512] with
            # xT[p, k, s] = x[r0 + s, 128k + p]
            r0 = bi * S + t * ST
            xT = xt_pool.tile([128, KE, ST], F16, tag="xT")
            nc.sync.dma_start_transpose(xT[:], x_d[r0:r0 + ST, :])
            return xT

        def dma_xb(bi, t):
            # straight fp16 copy for the pooling moving operand:
            # xb[p, c, e] = x[r0 + 128c + p, e]
            r0 = bi * S + t * ST
            xb = xb_pool.tile([128, NCH, E], F16, tag="xb")
            nc.sync.dma_start(
                xb[:], x_d[r0:r0 + ST, :].rearrange("(c p) e -> p c e", p=128))
            return xb

        # ---- online softmax + pooling, one sub-tile behind pass 1 ----

        def scores_copy(st):
            # DVE: raw scores PSUM -> SBUF row (the transposes a drain later
            # use it as a stationary operand, which must be SBUF)
            sc_row = sm_pool.tile([1, ST], F32, tag="srow")
            nc.vector.tensor_copy(sc_row[:], st["sc_ps"][:])
            st["sc_row"] = sc_row

        def softmax_partial(st):
            # PE: scores row [1,512] -> [128, 4] via 4 tiny transposes, then
            # GPSIMD cross-partition max + 128-wide exp; zcat partials stay
            # per-partition until the batch fixup
            t, sc_row = st["t"], st["sc_row"]
            tp = ps_t.tile([128, NCH], F32, tag="tps")
            for u in range(NCH):
                nc.tensor.matmul(
                    tp[:, u:u + 1], sc_row[0:1, u * 128:(u + 1) * 128],
                    ident[0:1, 0:1], is_transpose=True,
                    start=(u == 0), stop=(u == NCH - 1),
                    skip_group_check=True)
            mc = sm_pool.tile([128, 1], F32, tag="mc")
            nc.vector.reduce_max(mc[:], tp[:], axis=AX)
            nc.gpsimd.partition_all_reduce(st["mcat"][:, t:t + 1], mc[:], 128,
                                           ReduceOp.max)
            negm = sm_pool.tile([128, 1], F32, tag="negm")
            nc.vector.tensor_scalar_mul(negm[:], st["mcat"][:, t:t + 1], -1.0)
            pT = pt_pool.tile([128, NCH], F16, tag="pT")
            nc.scalar.activation(pT[:], tp[:], AF.Exp, bias=negm[:, 0:1],
                                 accum_out=st["zcat"][:, t:t + 1])
            st["pT"] = pT

        def pool_partial(st):
            # PE pooling of this sub-tile with its provisional weights
            pT, xb = st["pT"], st["xb"]
            op0 = ps_o.tile([1, 512], F32, tag="op0")
            op1 = ps_o.tile([1, 512], F32, tag="op1")
            for c in range(NCH):
                nc.tensor.matmul(op0[:], pT[:, c:c + 1], xb[:, c, 0:512],
                                 start=(c == 0), stop=(c == NCH - 1))
            for c in range(NCH):
                nc.tensor.matmul(op1[:], pT[:, c:c + 1], xb[:, c, 512:1024],
                                 start=(c == 0), stop=(c == NCH - 1))
            part = pp_pool.tile([1, E], F16, tag="part",
                                name=f"part{st['i']}")
            nc.scalar.activation(part[0:1, 0:512], op0[:], AF.Copy)
            nc.scalar.activation(part[0:1, 512:1024], op1[:], AF.Copy)
            st["parts"].append(part)

        def flush(st):
            # per-batch fixup: r_t = exp(m_t - m) / Z, all [128,*]-wide and
            # tiny; then rescale+sum the four [1, E] partial pools
            mcat, zcat = st["mcat"], st["zcat"]
            mG = sm_pool.tile([128, 1], F32, tag="mG")
            nc.vector.reduce_max(mG[:], mcat[:], axis=AX)
            negG = sm_pool.tile([128, 1], F32, tag="negG")
            nc.vector.tensor_scalar_mul(negG[:], mG[:], -1.0)
            r4 = sm_pool.tile([128, NSUB], F32, tag="r4")
            nc.scalar.activation(r4[:], mcat[:], AF.Exp, bias=negG[:, 0:1])
            rz4 = sm_pool.tile([128, NSUB], F32, tag="rz4")
            nc.vector.tensor_tensor(rz4[:], r4[:], zcat[:], ALU.mult)
            zp = sm_pool.tile([128, 1], F32, tag="zp")
            nc.vector.reduce_sum(zp[:], rz4[:], axis=AX)
            z_all = sm_pool.tile([128, 1], F32, tag="zall")
            nc.gpsimd.partition_all_reduce(z_all[:], zp[:], 128, ReduceOp.add)
            iz = sm_pool.tile([128, 1], F32, tag="iz")
            nc.vector.reciprocal(iz[:], z_all[:])
            rn4 = sm_pool.tile([128, NSUB], F32, tag="rn4")
            nc.vector.tensor_scalar_mul(rn4[:], r4[:], iz[:, 0:1])

            parts = st["parts"]
            sc = []
            for t in range(NSUB):
                s_t = sm_pool.tile([1, E], F16, tag=f"psc{t}")
                nc.vector.tensor_scalar_mul(s_t[:], parts[t][:],
                                            rn4[0:1, t:t + 1])
                sc.append(s_t)
            a01 = sm_pool.tile([1, E], F16, tag="a01")
            nc.vector.tensor_tensor(a01[:], sc[0][:], sc[1][:], ALU.add)
            a23 = sm_pool.tile([1, E], F16, tag="a23")
            nc.vector.tensor_tensor(a23[:], sc[2][:], sc[3][:], ALU.add)
            orow = out_pool.tile([1, E], F32, tag="orow")
            nc.vector.tensor_tensor(orow[:], a01[:], a23[:], ALU.add)
            nc.sync.dma_start(o_d[st["row"]:st["row"] + 1, :], orow[:])

        # prologue: first xT transposes go out before the W load so the
        # first matmul group can start ASAP
        # NOTE: do NOT issue these on another engine's DGE queue — a DMA
        # instruction occupies its issuing queue until dispatch, so W loads
        # on e.g. the Activation queue block the first tanhs behind them
        # (head-of-line), starving PSUM recycling and stalling the PE ~20us.
        # W/b/ctx ride the GpSimd SWDGE queue: its transfers overlap the
        # xbar transposes on the sync queue (16 real DMA engines), and
        # GpSimd's own first op (the sub-tile-0 partition_all_reduce) isn't
        # needed until ~30us, so nothing blocks behind these. (The
        # Activation queue was tried and head-of-line-blocked the tanhs.)
        nc.gpsimd.dma_start(b_sb[:], b_d.rearrange("(j p) -> p j", p=128))
        nc.gpsimd.dma_start(ctx_sb[:], c_d.rearrange("(j p) -> p j", p=128))
        for k in range(KE):
            nc.gpsimd.dma_start(W_sb[:, k * A:(k + 1) * A],
                                W_d[k * 128:(k + 1) * 128, :])
        xT_cur = dma_xt(tiles[0][1], tiles[0][2])
        xT_next = dma_xt(tiles[1][1], tiles[1][2])
        xb_cur = dma_xb(tiles[0][1], tiles[0][2])

        # warm the PE HAM clock-gate with throwaway matmuls while the first
        # DMAs land (the PE would otherwise idle cold and re-throttle)
        warm_scratch = out_pool.tile([128, 512], F32, tag="warm")
        for w in range(28):
            wp = ps_h.tile([128, 128], F32, tag="hps", name=f"warm{w}")
            nc.tensor.matmul(wp[:], ident[:], ident[:], start=True, stop=True)
            if w % 14 == 13:
                nc.scalar.activation(warm_scratch[:, 0:128], wp[:], AF.Copy)

        # deferred-work queues, all drained at later sub-tiles' j==1 as ONE
        # contiguous block (every extra PE interruption point costs ~0.4us
        # of entry/exit on HW): ctx-dot at lag 1, score transposes at lag 2
        # (so their DVE copy of the scores row is long done), pooling at
        # lag 3 (so the exp'd weights are long done), the batch fixup at
        # j==3 after the last partial pool
        q1 = []
        scoreT_q, pool_q, flush_q = [], [], []
        batch = None

        def drain(q):
            while q:
                q.pop(0)()

        for i, (rep, bi, t) in enumerate(tiles):
            if t == 0:
                batch = {"row": bi,
                         "mcat": sc_pool.tile([128, NSUB], F32, tag="mcat",
                                              name=f"mcat{i}"),
                         "zcat": sc_pool.tile([128, NSUB], F32, tag="zcat",
                                              name=f"zcat{i}"),
                         "parts": []}

            # prefetch: transposed tile two sub-tiles ahead, straight tile
            # one ahead
            if i + 2 < len(tiles):
                xT_pre = dma_xt(tiles[i + 2][1], tiles[i + 2][2])
            else:
                xT_pre = None
            if i + 1 < len(tiles):
                xb_next = dma_xb(tiles[i + 1][1], tiles[i + 1][2])
            else:
                xb_next = None

            st = dict(batch, i=i, t=t, xb=xb_cur)
            sc_ps = ps_s.tile([1, ST], F32, tag="scps")
            st["sc_ps"] = sc_ps

            for j in range(KA):
                hp = ps_h.tile([128, ST], F32, tag="hps")
                for k in range(KE):
                    nc.tensor.matmul(
                        hp[:],
                        W_sb[:, k * A + j * 128: k * A + (j + 1) * 128],
                        xT_cur[:, k, :],
                        start=(k == 0), stop=(k == KE - 1))
                if j == 1:
                    drain(q1)
                    if len(scoreT_q) >= 2:
                        scoreT_q.pop(0)()
                    if len(pool_q) >= 3:
                        pool_q.pop(0)()
                elif j == 3:
                    while flush_q and flush_q[0][0] <= i:
                        flush_q.pop(0)[1]()
                h_sb = h_pool.tile([128, ST], F16, tag="h")
                nc.scalar.activation(h_sb[:], hp[:], AF.Tanh,
                                     bias=b_sb[:, j:j + 1])
                q1.append(
                    lambda j=j, h_sb=h_sb, sc_ps=sc_ps: nc.tensor.matmul(
                        sc_ps[:], ctx_sb[:, j:j + 1], h_sb[:],
                        start=(j == 0), stop=(j == KA - 1)))
            q1.append(lambda st=st: scores_copy(st))
            scoreT_q.append(lambda st=st: softmax_partial(st))
            pool_q.append(lambda st=st: pool_partial(st))
            if t == NSUB - 1:
                flush_q.append((i + 3, lambda st=st: flush(st)))

            xT_cur, xT_next = xT_next, xT_pre
            xb_cur = xb_next

        drain(q1)
        drain(scoreT_q)
        drain(pool_q)
        while flush_q:
            flush_q.pop(0)[1]()

    nc.compile()
    return nc


_NC_CACHE = None


def kernel(x, W, b, ctx):
    global _NC_CACHE
    if _NC_CACHE is None:
        _NC_CACHE = _build()
    nc = _NC_CACHE

    x16 = np.ascontiguousarray(np.asarray(x).astype(np.float16))
    W16 = np.ascontiguousarray(np.asarray(W).astype(np.float16))
    b = np.ascontiguousarray(np.asarray(b, dtype=np.float32))
    c16 = np.ascontiguousarray(np.asarray(ctx).astype(np.float16))

    in_maps = [
        {"x": x16[i * BL:(i + 1) * BL].reshape(BL * S, E), "W": W16, "b": b,
         "ctx": c16}
        for i in range(NCORES)
    ]
    res = run_bass_kernel_spmd(nc, in_maps, core_ids=list(range(NCORES)))
    return np.concatenate([res.results[i]["out"] for i in range(NCORES)],
                          axis=0)


if __name__ == "__main__":
    rng = np.random.default_rng(0)
    x = rng.standard_normal((B, S, E), dtype=np.float32)
    W = rng.standard_normal((E, A), dtype=np.float32) / np.sqrt(E)
    b = rng.standard_normal((A,), dtype=np.float32) * 0.01
    c = rng.standard_normal((A,), dtype=np.float32)
    out = kernel(x=x, W=W, b=b, ctx=c)
    print(out.shape, out.dtype)


# revision 56
# speedup vs baseline: 1.0334x; 1.0334x over previous
"""Attention-pooling kernel for TRN2 (8 NeuronCores, batch-parallel).

Computes, for x:[32,2048,1024], W:[1024,1024], b:[1024], ctx:[1024]:
    h = tanh(x @ W + b); scores = h . ctx
    weights = softmax(scores, axis=seq)
    out = sum_s weights[s] * x[s]          -> [32, 1024]

Sharding: data-parallel over batch, 4 batches per core.

x and W are cast to fp16 on the host (10-bit mantissa keeps the score
error at the f32r-baseline level) so the kernel can use the xbar DMA
transpose: each 512-row seq sub-tile of x is transposed HBM -> SBUF by
the DMA engines directly into a [128, 8, 512] tile with
xT[p, k, s] = x[s, 128k + p], i.e. each embed-block k is a contiguous
moving operand for the PE — no PE transposes and no PSUM-evacuation
copies for pass 1.

All heavy math stays on the PE: offloading the pooling to DVE was
tried and REGRESSED (~20% slower matmuls): big [128,512] DVE reads
contend with the PE's moving-operand SBUF fetches, so idle engines are
not free. Instead the softmax+pooling is ONLINE (flash-attention
style): each sub-tile's scores are transposed (4 tiny PE matmuls),
exponentiated against the sub-tile max (GPSIMD cross-partition max,
128-wide ACT exp), and pooled immediately (8 PE matmuls against a
straight fp16 copy of x), one sub-tile behind pass 1. A per-batch
scalar fixup r_t = exp(m_t - m)/Z rescales the four partial pools, so
nothing big serializes behind the last scores. Mid-stream PE
stationary/bank switches cost ~2x95ns on HW, so deferred work drains
as contiguous same-shape groups between pass-1 matmul groups.
"""

import numpy as np
from contextlib import ExitStack

import concourse.bacc as bacc
import concourse.mybir as mybir
import concourse.tile as tile
from concourse import masks
from concourse.bass_isa import ReduceOp
from concourse.bass_utils import run_bass_kernel_spmd

B, S, E, A = 32, 2048, 1024, 1024
NCORES = 8
BL = B // NCORES          # batches per core
ST = 512                  # seq sub-tile
NSUB = S // ST            # sub-tiles per batch
NCH = ST // 128           # 128-row s-chunks per sub-tile
KE = E // 128             # contraction chunks over embed dim
KA = A // 128             # chunks over attention dim

F32 = mybir.dt.float32
F16 = mybir.dt.float16
AX = mybir.AxisListType.X
AF = mybir.ActivationFunctionType
ALU = mybir.AluOpType


def _build(reps=1):
    nc = bacc.Bacc("TRN2", target_bir_lowering=False, debug=False,
                   num_devices=NCORES)
    x_d = nc.declare_dram_parameter("x", [BL * S, E], F16, isOutput=False)
    W_d = nc.declare_dram_parameter("W", [E, A], F16, isOutput=False)
    b_d = nc.declare_dram_parameter("b", [A], F32, isOutput=False)
    c_d = nc.declare_dram_parameter("ctx", [A], F16, isOutput=False)
    o_d = nc.declare_dram_parameter("out", [BL, E], F32, isOutput=True)

    with ExitStack() as ctx:
        tc = ctx.enter_context(tile.TileContext(nc))

        const_pool = ctx.enter_context(tc.tile_pool(name="const", bufs=1))
        xt_pool = ctx.enter_context(tc.tile_pool(name="xT", bufs=3))
        xb_pool = ctx.enter_context(tc.tile_pool(name="xb", bufs=3))
        h_pool = ctx.enter_context(tc.tile_pool(name="h", bufs=12))
        sc_pool = ctx.enter_context(tc.tile_pool(name="scores", bufs=2))
        sm_pool = ctx.enter_context(tc.tile_pool(name="softmax", bufs=2))
        pp_pool = ctx.enter_context(tc.tile_pool(name="parts", bufs=5))
        pt_pool = ctx.enter_context(tc.tile_pool(name="pT", bufs=3))
        out_pool = ctx.enter_context(tc.tile_pool(name="outs", bufs=2))

        ps_h = ctx.enter_context(tc.tile_pool(name="ps_h", bufs=2, space="PSUM"))
        ps_s = ctx.enter_context(tc.tile_pool(name="ps_s", bufs=2, space="PSUM"))
        ps_t = ctx.enter_context(tc.tile_pool(name="ps_t", bufs=1, space="PSUM"))
        ps_o = ctx.enter_context(tc.tile_pool(name="ps_o", bufs=1, space="PSUM"))

        # ---- constants ----
        ident = const_pool.tile([128, 128], F32)
        masks.make_identity(nc, ident[:])

        W_sb = const_pool.tile([128, KE * A], F16)
        b_sb = const_pool.tile([128, KA], F32)
        ctx_sb = const_pool.tile([128, KA], F16)

        tiles = [(rep, bi, t)
                 for rep in range(reps) for bi in range(BL) for t in range(NSUB)]

        def dma_xt(bi, t):
            # xbar DMA transpose: x rows [512, 1024] -> [128, 8, 512] with
            # xT[p, k, s] = x[r0 + s, 128k + p]
            r0 = bi * S + t * ST
            xT = xt_pool.tile([128, KE, ST], F16, tag="xT")
            nc.sync.dma_start_transpose(xT[:], x_d[r0:r0 + ST, :])
            return xT

        def dma_xb2(bi, tpair):
            # straight fp16 copy for the pooling moving operand, loaded at
            # sub-tile-PAIR granularity (halves the DMA instruction count):
            # xb[p, c, e] = x[r0 + 128c + p, e], c in [0, 8)
            r0 = bi * S + tpair * 2 * ST
            xb = xb_pool.tile([128, 2 * NCH, E], F16, tag="xb")
            nc.sync.dma_start(
                xb[:], x_d[r0:r0 + 2 * ST, :].rearrange(
                    "(c p) e -> p c e", p=128))
            return xb

        # ---- online softmax + pooling, one sub-tile behind pass 1 ----

        def scores_copy(st):
            # DVE: raw scores PSUM -> SBUF row (the transposes a drain later
            # use it as a stationary operand, which must be SBUF)
            sc_row = sm_pool.tile([1, ST], F32, tag="srow")
            nc.vector.tensor_copy(sc_row[:], st["sc_ps"][:])
            st["sc_row"] = sc_row

        def softmax_partial(st):
            # PE: scores row [1,512] -> [128, 4] via 4 tiny transposes, then
            # GPSIMD cross-partition max + 128-wide exp; zcat partials stay
            # per-partition until the batch fixup
            t, sc_row = st["t"], st["sc_row"]
            tp = ps_t.tile([128, NCH], F32, tag="tps")
            for u in range(NCH):
                nc.tensor.matmul(
                    tp[:, u:u + 1], sc_row[0:1, u * 128:(u + 1) * 128],
                    ident[0:1, 0:1], is_transpose=True,
                    start=(u == 0), stop=(u == NCH - 1),
                    skip_group_check=True)
            mc = sm_pool.tile([128, 1], F32, tag="mc")
            nc.vector.reduce_max(mc[:], tp[:], axis=AX)
            nc.gpsimd.partition_all_reduce(st["mcat"][:, t:t + 1], mc[:], 128,
                                           ReduceOp.max)
            negm = sm_pool.tile([128, 1], F32, tag="negm")
            nc.vector.tensor_scalar_mul(negm[:], st["mcat"][:, t:t + 1], -1.0)
            pT = pt_pool.tile([128, NCH], F16, tag="pT")
            nc.scalar.activation(pT[:], tp[:], AF.Exp, bias=negm[:, 0:1],
                                 accum_out=st["zcat"][:, t:t + 1])
            st["pT"] = pT

        def pool_partial(st):
            # PE pooling of this sub-tile with its provisional weights
            pT, xb, off = st["pT"], st["xb"], st["xboff"]
            op0 = ps_o.tile([1, 512], F32, tag="op0")
            op1 = ps_o.tile([1, 512], F32, tag="op1")
            for c in range(NCH):
                nc.tensor.matmul(op0[:], pT[:, c:c + 1],
                                 xb[:, off + c, 0:512],
                                 start=(c == 0), stop=(c == NCH - 1))
            for c in range(NCH):
                nc.tensor.matmul(op1[:], pT[:, c:c + 1],
                                 xb[:, off + c, 512:1024],
                                 start=(c == 0), stop=(c == NCH - 1))
            part = pp_pool.tile([1, E], F16, tag="part",
                                name=f"part{st['i']}")
            nc.scalar.activation(part[0:1, 0:512], op0[:], AF.Copy)
            nc.scalar.activation(part[0:1, 512:1024], op1[:], AF.Copy)
            st["parts"].append(part)

        def flush(st):
            # per-batch fixup: r_t = exp(m_t - m) / Z, all [128,*]-wide and
            # tiny; then rescale+sum the four [1, E] partial pools
            mcat, zcat = st["mcat"], st["zcat"]
            mG = sm_pool.tile([128, 1], F32, tag="mG")
            nc.vector.reduce_max(mG[:], mcat[:], axis=AX)
            negG = sm_pool.tile([128, 1], F32, tag="negG")
            nc.vector.tensor_scalar_mul(negG[:], mG[:], -1.0)
            r4 = sm_pool.tile([128, NSUB], F32, tag="r4")
            nc.scalar.activation(r4[:], mcat[:], AF.Exp, bias=negG[:, 0:1])
            rz4 = sm_pool.tile([128, NSUB], F32, tag="rz4")
            nc.vector.tensor_tensor(rz4[:], r4[:], zcat[:], ALU.mult)
            zp = sm_pool.tile([128, 1], F32, tag="zp")
            nc.vector.reduce_sum(zp[:], rz4[:], axis=AX)
            z_all = sm_pool.tile([128, 1], F32, tag="zall")
            nc.gpsimd.partition_all_reduce(z_all[:], zp[:], 128, ReduceOp.add)
            iz = sm_pool.tile([128, 1], F32, tag="iz")
            nc.vector.reciprocal(iz[:], z_all[:])
            rn4 = sm_pool.tile([128, NSUB], F32, tag="rn4")
            nc.vector.tensor_scalar_mul(rn4[:], r4[:], iz[:, 0:1])

            parts = st["parts"]
            sc = []
            for t in range(NSUB):
                s_t = sm_pool.tile([1, E], F16, tag=f"psc{t}")
                nc.vector.tensor_scalar_mul(s_t[:], parts[t][:],
                                            rn4[0:1, t:t + 1])
                sc.append(s_t)
            a01 = sm_pool.tile([1, E], F16, tag="a01")
            nc.vector.tensor_tensor(a01[:], sc[0][:], sc[1][:], ALU.add)
            a23 = sm_pool.tile([1, E], F16, tag="a23")
            nc.vector.tensor_tensor(a23[:], sc[2][:], sc[3][:], ALU.add)
            orow = out_pool.tile([1, E], F32, tag="orow")
            nc.vector.tensor_tensor(orow[:], a01[:], a23[:], ALU.add)
            nc.sync.dma_start(o_d[st["row"]:st["row"] + 1, :], orow[:])

        # prologue: first xT transposes go out before the W load so the
        # first matmul group can start ASAP
        # NOTE: do NOT issue these on another engine's DGE queue — a DMA
        # instruction occupies its issuing queue until dispatch, so W loads
        # on e.g. the Activation queue block the first tanhs behind them
        # (head-of-line), starving PSUM recycling and stalling the PE ~20us.
        nc.sync.dma_start(b_sb[:], b_d.rearrange("(j p) -> p j", p=128))
        nc.sync.dma_start(ctx_sb[:], c_d.rearrange("(j p) -> p j", p=128))
        nc.sync.dma_start(W_sb[:, 0:A], W_d[0:128, :])
        nc.sync.dma_start(W_sb[:, A:2 * A], W_d[128:256, :])
        xT_cur = dma_xt(tiles[0][1], tiles[0][2])
        for k in range(2, KE):
            nc.sync.dma_start(W_sb[:, k * A:(k + 1) * A],
                              W_d[k * 128:(k + 1) * 128, :])
        xT_next = dma_xt(tiles[1][1], tiles[1][2])
        xb_cur = dma_xb2(tiles[0][1], 0)

        # warm the PE HAM clock-gate with throwaway matmuls while the first
        # DMAs land (the PE would otherwise idle cold and re-throttle)
        warm_scratch = out_pool.tile([128, 512], F32, tag="warm")
        for w in range(28):
            wp = ps_h.tile([128, 128], F32, tag="hps", name=f"warm{w}")
            nc.tensor.matmul(wp[:], ident[:], ident[:], start=True, stop=True)
            if w % 14 == 13:
                nc.scalar.activation(warm_scratch[:, 0:128], wp[:], AF.Copy)

        # deferred-work queues, all drained at later sub-tiles' j==1 as ONE
        # contiguous block (every extra PE interruption point costs ~0.4us
        # of entry/exit on HW): ctx-dot at lag 1, score transposes at lag 2
        # (so their DVE copy of the scores row is long done), pooling at
        # lag 3 (so the exp'd weights are long done), the batch fixup at
        # j==3 after the last partial pool
        q1 = []
        scoreT_q, pool_q, flush_q = [], [], []
        batch = None

        def drain(q):
            while q:
                q.pop(0)()

        for i, (rep, bi, t) in enumerate(tiles):
            if t == 0:
                batch = {"row": bi,
                         "mcat": sc_pool.tile([128, NSUB], F32, tag="mcat",
                                              name=f"mcat{i}"),
                         "zcat": sc_pool.tile([128, NSUB], F32, tag="zcat",
                                              name=f"zcat{i}"),
                         "parts": []}

            # prefetch: transposed tile two sub-tiles ahead, straight tile
            # one ahead
            if i + 2 < len(tiles):
                xT_pre = dma_xt(tiles[i + 2][1], tiles[i + 2][2])
            else:
                xT_pre = None
            if i % 2 == 0 and i + 2 < len(tiles):
                xb_next = dma_xb2(tiles[i + 2][1], tiles[i + 2][2] // 2)

            st = dict(batch, i=i, t=t, xb=xb_cur, xboff=(i % 2) * NCH)
            sc_ps = ps_s.tile([1, ST], F32, tag="scps")
            st["sc_ps"] = sc_ps

            for j in range(KA):
                hp = ps_h.tile([128, ST], F32, tag="hps")
                for k in range(KE):
                    nc.tensor.matmul(
                        hp[:],
                        W_sb[:, k * A + j * 128: k * A + (j + 1) * 128],
                        xT_cur[:, k, :],
                        start=(k == 0), stop=(k == KE - 1))
                if j == 1:
                    drain(q1)
                    if len(scoreT_q) >= 2:
                        scoreT_q.pop(0)()
                    if len(pool_q) >= 3:
                        pool_q.pop(0)()
                elif j == 3:
                    while flush_q and flush_q[0][0] <= i:
                        flush_q.pop(0)[1]()
                h_sb = h_pool.tile([128, ST], F16, tag="h")
                nc.scalar.activation(h_sb[:], hp[:], AF.Tanh,
                                     bias=b_sb[:, j:j + 1])
                q1.append(
                    lambda j=j, h_sb=h_sb, sc_ps=sc_ps: nc.tensor.matmul(
                        sc_ps[:], ctx_sb[:, j:j + 1], h_sb[:],
                        start=(j == 0), stop=(j == KA - 1)))
            q1.append(lambda st=st: scores_copy(st))
            scoreT_q.append(lambda st=st: softmax_partial(st))
            pool_q.append(lambda st=st: pool_partial(st))
            if t == NSUB - 1:
                flush_q.append((i + 3, lambda st=st: flush(st)))

            xT_cur, xT_next = xT_next, xT_pre
            if i % 2 == 1:
                xb_cur = xb_next

        drain(q1)
        drain(scoreT_q)
        drain(pool_q)
        while flush_q:
            flush_q.pop(0)[1]()

    nc.compile()
    return nc


_NC_CACHE = None


def kernel(x, W, b, ctx):
    global _NC_CACHE
    if _NC_CACHE is None:
        _NC_CACHE = _build()
    nc = _NC_CACHE

    x16 = np.ascontiguousarray(np.asarray(x).astype(np.float16))
    W16 = np.ascontiguousarray(np.asarray(W).astype(np.float16))
    b = np.ascontiguousarray(np.asarray(b, dtype=np.float32))
    c16 = np.ascontiguousarray(np.asarray(ctx).astype(np.float16))

    in_maps = [
        {"x": x16[i * BL:(i + 1) * BL].reshape(BL * S, E), "W": W16, "b": b,
         "ctx": c16}
        for i in range(NCORES)
    ]
    res = run_bass_kernel_spmd(nc, in_maps, core_ids=list(range(NCORES)))
    return np.concatenate([res.results[i]["out"] for i in range(NCORES)],
                          axis=0)


if __name__ == "__main__":
    rng = np.random.default_rng(0)
    x = rng.standard_normal((B, S, E), dtype=np.float32)
    W = rng.standard_normal((E, A), dtype=np.float32) / np.sqrt(E)
    b = rng.standard_normal((A,), dtype=np.float32) * 0.01
    c = rng.standard_normal((A,), dtype=np.float32)
    out = kernel(x=x, W=W, b=b, ctx=c)
    print(out.shape, out.dtype)
